# revision 14
# baseline (speedup 1.0000x reference)
# Trainium2 Bass kernel for nn_AttentionLayer (BiDAF-style attention).
#
# Math (T=16384, J=1024, D2=512):
#   w1,w2,w3 = Ws blocks;  S[t,j] = H@w1 + U@w2 + (H*w3)@U.T
#   A  = softmax_j(S) @ U                      (C2Q)
#   b  = softmax_t(max_j S);  h~ = b @ H       (Q2C, global over T)
#   G  = [H | A | H*A | H*h~]                  (T, 2048)
#
# Sharding: T rows split across 8 cores (2048 rows each). U/Ws replicated.
# Per core everything is local except (hnum = sum_t exp(m_t) H_t, ssum =
# sum_t exp(m_t)) which are AllReduce'd (513 floats).  A dummy AllReduce
# fires at kernel start to absorb the ~70us first-collective warmup.
#
# Layout trick: compute S^T tiles [j_part, t_free] so the C2Q attend matmul
# (A = P @ U) can use E=exp(S^T) slices directly as the stationary operand.
# exp bias handles the s2[j] term (per-partition); the s1[t] term cancels in
# softmax_j and is reapplied only to the Q2C row maxima.  s1/s2 themselves
# are computed on the Vector engine (tensor_tensor_reduce against
# host-broadcast w1/w2 rows) straight into per-partition columns -- the PE
# is the bottleneck engine, so everything small is pushed off it.
#
# Matmuls run as float32r (TF32-like: ~1.5e-4 rel err, ~3x faster than fp32).

import numpy as np

T, J, D2 = 16384, 1024, 512
NCORES = 8
TC = T // NCORES            # 2048 context rows per core
NCHUNK = 4                  # t-chunks per core
CHUNK = TC // NCHUNK        # 512
NTT = TC // 128             # 16 t-tiles per core
NJT = J // 128              # 8 j-tiles
NKT = D2 // 128             # 4 d-tiles

MM_BF16 = False             # bf16 for the S / A matmul operands

_CACHE = {}
LAST = {}


def _build_nc():
    import concourse.bacc as bacc
    import concourse.mybir as mybir
    import concourse.tile as tile

    f32 = mybir.dt.float32
    f32r = mybir.dt.float32r
    bf16 = mybir.dt.bfloat16
    mmdt = bf16 if MM_BF16 else f32r
    X = mybir.AxisListType.X
    MAX = mybir.AluOpType.max
    ADD = mybir.AluOpType.add
    MULT = mybir.AluOpType.mult
    EXP = mybir.ActivationFunctionType.Exp

    def f(ap):  # view an fp32r AP as plain fp32 for non-matmul consumers
        return ap.bitcast(f32) if ap.dtype == f32r else ap

    nc = bacc.Bacc("TRN2", target_bir_lowering=False, debug=False,
                   num_devices=NCORES)

    HT = nc.dram_tensor("HT", [D2, TC], mmdt, kind="ExternalInput")
    Hn = nc.dram_tensor("Hn", [TC, D2], f32r, kind="ExternalInput")
    Un = nc.dram_tensor("Un", [J, D2], mmdt, kind="ExternalInput")
    UW = nc.dram_tensor("UW", [D2, J], mmdt, kind="ExternalInput")
    W1b = nc.dram_tensor("W1b", [128, D2], f32, kind="ExternalInput")
    W2b = nc.dram_tensor("W2b", [128, D2], f32, kind="ExternalInput")
    Id = nc.dram_tensor("Id", [128, 128], f32, kind="ExternalInput")
    On = nc.dram_tensor("On", [1, 128], f32r, kind="ExternalInput")
    Oc = nc.dram_tensor("Oc", [128, 2], f32r, kind="ExternalInput")
    G = nc.dram_tensor("G", [TC, 4 * D2], f32, kind="ExternalOutput")

    with tile.TileContext(nc) as tc:
        with (
            tc.tile_pool(name="persist", bufs=1) as pp,
            tc.tile_pool(name="stream", bufs=2) as sp,
            tc.tile_pool(name="stage", bufs=4) as gp,
            tc.tile_pool(name="spsum", bufs=2, space="PSUM") as spsum,
            tc.tile_pool(name="apsum", bufs=2, space="PSUM") as apsum,
            tc.tile_pool(name="trpsum", bufs=1, space="PSUM") as trpsum,
            tc.tile_pool(name="rowpsum", bufs=1, space="PSUM") as rowpsum,
            tc.tile_pool(name="dram", bufs=1, space="DRAM") as dram,
        ):
            # ---- dummy collective first: pays the ~70us first-collective
            # warmup on TOPSP/SDMA while the engines do real work.
            dummy_sb = pp.tile([1, 8], f32, tag="dummy_sb")
            nc.vector.memset(dummy_sb[:], 0.0)
            dummy_in = dram.tile([1, 8], f32, tag="dummy_in")
            dummy_out = dram.tile([1, 8], f32, tag="dummy_out")
            nc.sync.dma_start(dummy_in[:], dummy_sb[:])
            nc.gpsimd.collective_compute(
                "AllReduce", ADD, replica_groups=[list(range(NCORES))],
                ins=[dummy_in.opt()], outs=[dummy_out.opt()],
            )

            # ---- loads, in the order the PE needs them: the first S matmul
            # wants uw3[kt] + ht[kt, chunk0]
            uw3 = pp.tile([128, NKT, J], mmdt, tag="uw3")
            ht = pp.tile([128, NKT, TC], mmdt, tag="ht")
            for kt in range(NKT):
                nc.sync.dma_start(
                    uw3[:, kt, :],
                    UW.ap()[kt * 128:(kt + 1) * 128, :])
                nc.sync.dma_start(
                    ht[:, kt, 0:CHUNK],
                    HT.ap()[kt * 128:(kt + 1) * 128, 0:CHUNK])
            for c in range(1, NCHUNK):
                cs, ce = c * CHUNK, (c + 1) * CHUNK
                nc.sync.dma_start(
                    ht[:, :, cs:ce],
                    HT.ap()[:, cs:ce].rearrange("(kt p) t -> p kt t", p=128))
            un = pp.tile([128, NJT, D2], mmdt, tag="un")
            nc.sync.dma_start(un[:], Un.ap().rearrange("(jt p) d -> p jt d", p=128))
            w2b = pp.tile([128, D2], f32, tag="w2b")
            nc.sync.dma_start(w2b[:], W2b.ap()[:])
            w1b = pp.tile([128, D2], f32, tag="w1b")
            nc.sync.dma_start(w1b[:], W1b.ap()[:])
            ident = pp.tile([128, 128], f32, tag="ident")
            nc.sync.dma_start(ident[:], Id.ap()[:])
            hn = pp.tile([128, NTT, D2], f32r, tag="hn")
            for c in range(NCHUNK):
                cs, ce = c * CHUNK, (c + 1) * CHUNK
                nc.sync.dma_start(
                    hn[:, 4 * c:4 * (c + 1), :],
                    Hn.ap()[cs:ce, :].rearrange("(tt p) d -> p tt d", p=128))
            onesrow = pp.tile([1, 128], f32r, tag="onesrow")
            nc.sync.dma_start(onesrow[:], On.ap()[:])
            onescol = pp.tile([128, 2], f32r, tag="onescol")
            nc.sync.dma_start(onescol[:], Oc.ap()[:])

            # ---- s2[j] = U @ w2 on DVE: per-(j)-partition columns directly
            s2col = pp.tile([128, NJT], f32, tag="s2col")
            for jt in range(NJT):
                scr = gp.tile([128, D2], f32, tag="ttscr")
                nc.vector.tensor_tensor(scr[:], f(un[:, jt, :]), w2b[:], MULT)
                nc.vector.tensor_reduce(s2col[:, jt:jt + 1], scr[:], X, ADD)

            # ---- persistent accumulators
            emax = pp.tile([128, NTT], f32, tag="emax")    # max_j E'' per t
            s1col = pp.tile([128, NTT], f32, tag="s1col")  # s1[t]
            es1 = pp.tile([128, NTT], f32, tag="es1")      # exp(s1[t])
            bnum = pp.tile([128, NTT], f32r, tag="bnum")   # exp(m[t])
            hnum_sb = pp.tile([1, D2], f32, tag="hnum_sb")  # sum_t bnum*H

            for c in range(NCHUNK):
                cs, ce = c * CHUNK, (c + 1) * CHUNK

                # S^T tiles for this chunk -> E'' = exp(S^T + s2[j]).
                # 4 interleaved PSUM chains: same-bank accumulation
                # serializes the PE (464 -> 300ns/pair measured).
                e = sp.tile([128, NJT, CHUNK], mmdt, tag="e")
                for jq in range(0, NJT, 2):
                    spss = [spsum.tile([128, CHUNK], f32, tag="sps",
                                       name=f"sps_{c}_{jq}_{q}")
                            for q in range(2)]
                    for kt in range(NKT):
                        for q in range(2):
                            nc.tensor.matmul(
                                spss[q][:],
                                uw3[:, kt, (jq + q) * 128:(jq + q + 1) * 128],
                                ht[:, kt, cs:ce],
                                start=(kt == 0), stop=(kt == NKT - 1))
                    for q in range(2):
                        nc.scalar.activation(e[:, jq + q, :], spss[q][:], EXP,
                                             bias=s2col[:, jq + q:jq + q + 1])

                # partial max/sum over the 8 j-tiles (chained tensor_tensor)
                pmax = sp.tile([128, CHUNK], f32, tag="pmax")
                psm = sp.tile([128, CHUNK], f32r, tag="psm")
                nc.vector.tensor_tensor(pmax[:], f(e[:, 0, :]), f(e[:, 1, :]), MAX)
                nc.vector.tensor_tensor(psm[:], f(e[:, 0, :]), f(e[:, 1, :]), ADD)
                for jt in range(2, NJT):
                    nc.vector.tensor_tensor(pmax[:], pmax[:], f(e[:, jt, :]), MAX)
                    nc.vector.tensor_tensor(psm[:], f(psm[:]), f(e[:, jt, :]), ADD)

                # s1[t] columns for the 4 t-tiles of this chunk (DVE)
                for i in range(4):
                    tt = 4 * c + i
                    scr = gp.tile([128, D2], f32, tag="ttscr")
                    nc.vector.tensor_tensor(scr[:], f(hn[:, tt, :]), w1b[:],
                                            MULT)
                    nc.vector.tensor_reduce(s1col[:, tt:tt + 1], scr[:], X,
                                            ADD)
                    nc.scalar.activation(es1[:, tt:tt + 1],
                                         s1col[:, tt:tt + 1], EXP)

                hnps = rowpsum.tile([1, D2], f32, tag="row")
                for ip in range(0, 4, 2):
                    pair = (ip, ip + 1)
                    dpss = {}
                    for i in pair:
                        tt = 4 * c + i
                        # emax: transpose pmax 128-block, reduce over parts
                        tpm = trpsum.tile([128, 128], f32, tag="tr",
                                          name=f"tpm_{c}_{i}")
                        nc.tensor.transpose(tpm[:],
                                            pmax[:, i * 128:(i + 1) * 128],
                                            ident[:])
                        nc.vector.tensor_reduce(emax[:, tt:tt + 1], tpm[:],
                                                X, MAX)
                        # dcol: ones-matmul (cheaper than transpose+reduce)
                        dps = trpsum.tile([128, 2], f32, tag="dcol",
                                          name=f"dps_{c}_{i}")
                        nc.tensor.matmul(dps[:],
                                         psm[:, i * 128:(i + 1) * 128],
                                         onescol[:], start=True, stop=True)
                        dpss[i] = dps
                        # bnum = exp(m[t]) = emax * exp(s1)
                        nc.vector.tensor_tensor(bnum[:, tt:tt + 1],
                                                emax[:, tt:tt + 1],
                                                es1[:, tt:tt + 1], MULT)
                        # Q2C numerator: hnps += bnum_tile.T @ H_tile
                        nc.tensor.matmul(hnps[:], bnum[:, tt:tt + 1],
                                         hn[:, tt, :],
                                         start=(i == 0), stop=(i == 3))

                    # C2Q attend: A = (E''.T @ U) / D, 2 interleaved chains
                    apss = [apsum.tile([128, D2], f32, tag="aps",
                                       name=f"aps_{c}_{ip}_{q}")
                            for q in range(2)]
                    for jt in range(NJT):
                        for q in range(2):
                            i = ip + q
                            nc.tensor.matmul(
                                apss[q][:],
                                e[:, jt, i * 128:(i + 1) * 128],
                                un[:, jt, :],
                                start=(jt == 0), stop=(jt == NJT - 1))
                    for q in range(2):
                        i = ip + q
                        tt = 4 * c + i
                        dinv = gp.tile([128, 1], f32, tag="dinv")
                        nc.vector.reciprocal(dinv[:], dpss[i][:, 0:1])
                        a_sb = gp.tile([128, D2], f32, tag="a_sb")
                        nc.vector.tensor_scalar_mul(a_sb[:], apss[q][:],
                                                    dinv[:])
                        ha_sb = gp.tile([128, D2], f32, tag="ha_sb")
                        eng = nc.vector if (tt % 2 == 0) else nc.gpsimd
                        eng.tensor_tensor(ha_sb[:], f(hn[:, tt, :]), a_sb[:],
                                          MULT)

                        ts_, te_ = tt * 128, (tt + 1) * 128
                        nc.sync.dma_start(G.ap()[ts_:te_, 0:D2],
                                          f(hn[:, tt, :]))
                        nc.sync.dma_start(G.ap()[ts_:te_, D2:2 * D2], a_sb[:])
                        nc.sync.dma_start(G.ap()[ts_:te_, 2 * D2:3 * D2],
                                          ha_sb[:])

                # fold chunk's Q2C numerator into SBUF accumulator
                if c == 0:
                    nc.vector.tensor_copy(hnum_sb[:], hnps[:])
                else:
                    nc.vector.tensor_tensor(hnum_sb[:], hnum_sb[:], hnps[:], ADD)

            # ---- Q2C global: AllReduce(hnum | ssum)
            ssps = rowpsum.tile([1, NTT], f32, tag="row")
            nc.tensor.matmul(ssps[:], onescol[:, 0:1], bnum[:],
                             start=True, stop=True)
            arow = pp.tile([1, 520], f32, tag="arow")
            nc.vector.memset(arow[:], 0.0)
            nc.vector.tensor_copy(arow[0:1, 0:D2], hnum_sb[:])
            nc.vector.tensor_reduce(arow[0:1, D2:D2 + 1], ssps[:], X, ADD)
            ar_in = dram.tile([1, 520], f32, tag="ar_in")
            ar_out = dram.tile([1, 520], f32, tag="ar_out")
            nc.sync.dma_start(ar_in[:], arow[:])
            nc.gpsimd.collective_compute(
                "AllReduce", ADD, replica_groups=[list(range(NCORES))],
                ins=[ar_in.opt()], outs=[ar_out.opt()],
            )
            hg = pp.tile([1, 520], f32, tag="hg")
            nc.sync.dma_start(hg[:], ar_out[:])

            # h~ = hnum_g / ssum_g, broadcast to all partitions
            zinv = pp.tile([1, 1], f32, tag="zinv")
            nc.vector.reciprocal(zinv[:], hg[0:1, D2:D2 + 1])
            htrow = pp.tile([1, D2], f32r, tag="htrow")
            nc.vector.tensor_scalar_mul(htrow[:], hg[0:1, 0:D2], zinv[:])
            htps = apsum.tile([128, D2], f32, tag="aps")
            nc.tensor.matmul(htps[:], onesrow[:], htrow[:],
                             start=True, stop=True)
            hts = pp.tile([128, D2], f32, tag="hts")
            nc.vector.tensor_copy(hts[:], htps[:])

            # G block 3: H * h~ (split DVE / GpSimd to shorten the tail)
            for tt in range(NTT):
                hh_sb = gp.tile([128, D2], f32, tag="hh_sb")
                eng = nc.vector if (tt % 2 == 0) else nc.gpsimd
                eng.tensor_tensor(hh_sb[:], f(hn[:, tt, :]), hts[:], MULT)
                nc.sync.dma_start(G.ap()[tt * 128:(tt + 1) * 128, 3 * D2:4 * D2],
                                  hh_sb[:])

    nc.compile()
    return nc


def kernel(H, U, Ws):
    import concourse.mybir as mybir
    from concourse import bass_utils

    H = np.ascontiguousarray(np.asarray(H, dtype=np.float32))
    U = np.ascontiguousarray(np.asarray(U, dtype=np.float32))
    Ws = np.asarray(Ws, dtype=np.float32)

    if "nc" not in _CACHE:
        _CACHE["nc"] = _build_nc()
    nc = _CACHE["nc"]

    mmnp = (mybir.dt.np(mybir.dt.bfloat16) if MM_BF16 else np.float32)

    w1 = Ws[0:D2, 0]
    w2 = Ws[D2:2 * D2, 0]
    w3 = Ws[2 * D2:3 * D2, 0]
    UW = np.ascontiguousarray(U.T * w3[:, None]).astype(mmnp)
    Unc = U.astype(mmnp)
    W1b = np.ascontiguousarray(np.broadcast_to(w1, (128, D2)))
    W2b = np.ascontiguousarray(np.broadcast_to(w2, (128, D2)))
    ident = np.eye(128, dtype=np.float32)

    in_maps = []
    for c in range(NCORES):
        Hc = H[c * TC:(c + 1) * TC]
        in_maps.append({
            "HT": np.ascontiguousarray(Hc.T).astype(mmnp),
            "Hn": Hc,
            "Un": Unc,
            "UW": UW,
            "W1b": W1b,
            "W2b": W2b,
            "Id": ident,
            "On": np.ones((1, 128), dtype=np.float32),
            "Oc": np.ones((128, 2), dtype=np.float32),
        })

    res = bass_utils.run_bass_kernel_spmd(
        nc, in_maps, core_ids=list(range(NCORES)))
    LAST["exec_time_ns"] = res.exec_time_ns
    G_full = np.concatenate([res.results[c]["G"] for c in range(NCORES)],
                            axis=0)
    return G_full.astype(np.float32, copy=False)


# revision 15
# speedup vs baseline: 1.1386x; 1.1386x over previous
# Trainium2 Bass kernel for nn_AttentionLayer (BiDAF-style attention).
#
# Math (T=16384, J=1024, D2=512):
#   w1,w2,w3 = Ws blocks;  S[t,j] = H@w1 + U@w2 + (H*w3)@U.T
#   A  = softmax_j(S) @ U                      (C2Q)
#   b  = softmax_t(max_j S);  h~ = b @ H       (Q2C, global over T)
#   G  = [H | A | H*A | H*h~]                  (T, 2048)
#
# Sharding: T rows split across 8 cores (2048 rows each). U/Ws replicated.
# Per core everything is local except (hnum = sum_t exp(m_t) H_t, ssum =
# sum_t exp(m_t)) which are AllReduce'd (513 floats).  A dummy AllReduce
# fires at kernel start to absorb the ~70us first-collective warmup.
#
# Layout trick: compute S^T tiles [j_part, t_free] so the C2Q attend matmul
# (A = P @ U) can use E=exp(S^T) slices directly as the stationary operand.
# exp bias handles the s2[j] term (per-partition); the s1[t] term cancels in
# softmax_j and is reapplied only to the Q2C row maxima.
#
# Perf structure (PE is the bottleneck engine):
#  - matmuls in float32r (TF32-like, ~3x plain fp32)
#  - S and A accumulations run as two interleaved PSUM chains: same-bank
#    accumulation serializes the PE (measured 464 -> 300ns per LDW+MM pair)
#  - per chunk: phase 1 = S matmuls + exp; phase 2 = Q2C-critical
#    reductions (max/sum partials, bnum, hnum); phase 3 = A matmuls +
#    G writes.  The final AllReduce is emitted before chunk 3's phase 3 so
#    it overlaps the remaining C2Q work.

import numpy as np

T, J, D2 = 16384, 1024, 512
NCORES = 8
TC = T // NCORES            # 2048 context rows per core
NCHUNK = 4                  # t-chunks per core
CHUNK = TC // NCHUNK        # 512
NTT = TC // 128             # 16 t-tiles per core
NJT = J // 128              # 8 j-tiles
NKT = D2 // 128             # 4 d-tiles

MM_BF16 = False             # bf16 for the S / A matmul operands

_CACHE = {}
LAST = {}


def _build_nc():
    import concourse.bacc as bacc
    import concourse.mybir as mybir
    import concourse.tile as tile

    f32 = mybir.dt.float32
    f32r = mybir.dt.float32r
    bf16 = mybir.dt.bfloat16
    mmdt = bf16 if MM_BF16 else f32r
    X = mybir.AxisListType.X
    MAX = mybir.AluOpType.max
    ADD = mybir.AluOpType.add
    MULT = mybir.AluOpType.mult
    EXP = mybir.ActivationFunctionType.Exp

    def f(ap):  # view an fp32r AP as plain fp32 for non-matmul consumers
        return ap.bitcast(f32) if ap.dtype == f32r else ap

    nc = bacc.Bacc("TRN2", target_bir_lowering=False, debug=False,
                   num_devices=NCORES)

    HT = nc.dram_tensor("HT", [D2, TC], mmdt, kind="ExternalInput")
    Hn = nc.dram_tensor("Hn", [TC, D2], f32r, kind="ExternalInput")
    Un = nc.dram_tensor("Un", [J, D2], mmdt, kind="ExternalInput")
    UW = nc.dram_tensor("UW", [D2, J], mmdt, kind="ExternalInput")
    Wc = nc.dram_tensor("Wc", [D2, 2], f32r, kind="ExternalInput")
    W2b = nc.dram_tensor("W2b", [128, D2], f32, kind="ExternalInput")
    Id = nc.dram_tensor("Id", [128, 128], f32, kind="ExternalInput")
    On = nc.dram_tensor("On", [1, 128], f32r, kind="ExternalInput")
    Oc = nc.dram_tensor("Oc", [128, 2], f32r, kind="ExternalInput")
    G = nc.dram_tensor("G", [TC, 4 * D2], f32, kind="ExternalOutput")

    with tile.TileContext(nc) as tc:
        with (
            tc.tile_pool(name="persist", bufs=1) as pp,
            tc.tile_pool(name="stream", bufs=2) as sp,
            tc.tile_pool(name="stage", bufs=4) as gp,
            tc.tile_pool(name="spsum", bufs=3, space="PSUM") as spsum,
            tc.tile_pool(name="apsum", bufs=2, space="PSUM") as apsum,
            tc.tile_pool(name="trpsum", bufs=1, space="PSUM") as trpsum,
            tc.tile_pool(name="rowpsum", bufs=1, space="PSUM") as rowpsum,
            tc.tile_pool(name="dram", bufs=1, space="DRAM") as dram,
        ):
            # ---- dummy collective first: pays the ~70us first-collective
            # warmup on TOPSP/SDMA while the engines do real work.
            dummy_sb = pp.tile([1, 8], f32, tag="dummy_sb")
            nc.vector.memset(dummy_sb[:], 0.0)
            dummy_in = dram.tile([1, 8], f32, tag="dummy_in")
            dummy_out = dram.tile([1, 8], f32, tag="dummy_out")
            nc.sync.dma_start(dummy_in[:], dummy_sb[:])
            nc.gpsimd.collective_compute(
                "AllReduce", ADD, replica_groups=[list(range(NCORES))],
                ins=[dummy_in.opt()], outs=[dummy_out.opt()],
            )

            # ---- loads, in the order the pipeline consumes them:
            # S matmuls need uw3 + ht[chunk0]; the first exps need s2col,
            # which needs un + w2b.
            uw3 = pp.tile([128, NKT, J], mmdt, tag="uw3")
            ht = pp.tile([128, NKT, TC], mmdt, tag="ht")
            nc.sync.dma_start(uw3[:, 0, :], UW.ap()[0:128, :])
            nc.sync.dma_start(ht[:, 0, 0:CHUNK], HT.ap()[0:128, 0:CHUNK])
            un = pp.tile([128, NJT, D2], mmdt, tag="un")
            nc.sync.dma_start(un[:], Un.ap().rearrange("(jt p) d -> p jt d", p=128))
            w2b = pp.tile([128, D2], f32, tag="w2b")
            nc.sync.dma_start(w2b[:], W2b.ap()[:])
            for kt in range(1, NKT):
                nc.sync.dma_start(
                    uw3[:, kt, :], UW.ap()[kt * 128:(kt + 1) * 128, :])
                nc.sync.dma_start(
                    ht[:, kt, 0:CHUNK],
                    HT.ap()[kt * 128:(kt + 1) * 128, 0:CHUNK])
            wcol = pp.tile([128, NKT, 2], f32r, tag="wcol")
            nc.sync.dma_start(wcol[:], Wc.ap().rearrange("(kt p) w -> p kt w", p=128))
            ident = pp.tile([128, 128], f32, tag="ident")
            nc.sync.dma_start(ident[:], Id.ap()[:])
            for c in range(1, NCHUNK):
                cs, ce = c * CHUNK, (c + 1) * CHUNK
                nc.sync.dma_start(
                    ht[:, :, cs:ce],
                    HT.ap()[:, cs:ce].rearrange("(kt p) t -> p kt t", p=128))
            hn = pp.tile([128, NTT, D2], f32r, tag="hn")
            for c in range(NCHUNK):
                cs, ce = c * CHUNK, (c + 1) * CHUNK
                nc.sync.dma_start(
                    hn[:, 4 * c:4 * (c + 1), :],
                    Hn.ap()[cs:ce, :].rearrange("(tt p) d -> p tt d", p=128))
            onesrow = pp.tile([1, 128], f32r, tag="onesrow")
            nc.sync.dma_start(onesrow[:], On.ap()[:])
            onescol = pp.tile([128, 2], f32r, tag="onescol")
            nc.sync.dma_start(onescol[:], Oc.ap()[:])

            # ---- s2[j] = U @ w2 on DVE: per-(j)-partition columns directly
            s2col = pp.tile([128, NJT], f32, tag="s2col")
            for jt in range(NJT):
                scr = gp.tile([128, D2], f32, tag="ttscr")
                nc.vector.tensor_tensor(scr[:], f(un[:, jt, :]), w2b[:], MULT)
                nc.vector.tensor_reduce(s2col[:, jt:jt + 1], scr[:], X, ADD)

            # ---- persistent accumulators
            emax = pp.tile([128, NTT], f32, tag="emax")    # max_j E'' per t
            dcol = pp.tile([128, NTT], f32, tag="dcol")    # sum_j E'' per t
            es1 = pp.tile([128, NTT], f32, tag="es1")      # exp(s1[t])
            bnum = pp.tile([128, NTT], f32r, tag="bnum")   # exp(m[t])
            hnum_sb = pp.tile([1, D2], f32, tag="hnum_sb")  # sum_t bnum*H

            def q2c_global():
                # ssum = sum_t bnum[t];  AllReduce([hnum | ssum])
                ssps = rowpsum.tile([1, NTT], f32, tag="row", name="ssps")
                nc.tensor.matmul(ssps[:], onescol[:, 0:1], bnum[:],
                                 start=True, stop=True)
                arow = pp.tile([1, 520], f32, tag="arow")
                nc.vector.memset(arow[:], 0.0)
                nc.vector.tensor_copy(arow[0:1, 0:D2], hnum_sb[:])
                nc.vector.tensor_reduce(arow[0:1, D2:D2 + 1], ssps[:], X, ADD)
                ar_in = dram.tile([1, 520], f32, tag="ar_in")
                ar_out = dram.tile([1, 520], f32, tag="ar_out")
                nc.sync.dma_start(ar_in[:], arow[:])
                nc.gpsimd.collective_compute(
                    "AllReduce", ADD, replica_groups=[list(range(NCORES))],
                    ins=[ar_in.opt()], outs=[ar_out.opt()],
                )
                hg = pp.tile([1, 520], f32, tag="hg")
                nc.sync.dma_start(hg[:], ar_out[:])
                # h~ = hnum_g / ssum_g, broadcast to all partitions
                zinv = pp.tile([1, 1], f32, tag="zinv")
                nc.vector.reciprocal(zinv[:], hg[0:1, D2:D2 + 1])
                htrow = pp.tile([1, D2], f32r, tag="htrow")
                nc.vector.tensor_scalar_mul(htrow[:], hg[0:1, 0:D2], zinv[:])
                htps = apsum.tile([128, D2], f32, tag="aps", name="htps")
                nc.tensor.matmul(htps[:], onesrow[:], htrow[:],
                                 start=True, stop=True)
                hts = pp.tile([128, D2], f32, tag="hts")
                nc.vector.tensor_copy(hts[:], htps[:])
                return hts

            hts = None
            for c in range(NCHUNK):
                cs, ce = c * CHUNK, (c + 1) * CHUNK

                # ---- phase 1: S^T tiles -> E'' = exp(S^T + s2[j]),
                # two interleaved PSUM chains
                e = sp.tile([128, NJT, CHUNK], mmdt, tag="e")
                for jq in range(0, NJT, 2):
                    spss = [spsum.tile([128, CHUNK], f32, tag="sps",
                                       name=f"sps_{c}_{jq}_{q}")
                            for q in range(2)]
                    for kt in range(NKT):
                        for q in range(2):
                            nc.tensor.matmul(
                                spss[q][:],
                                uw3[:, kt, (jq + q) * 128:(jq + q + 1) * 128],
                                ht[:, kt, cs:ce],
                                start=(kt == 0), stop=(kt == NKT - 1))
                    for q in range(2):
                        nc.scalar.activation(e[:, jq + q, :], spss[q][:], EXP,
                                             bias=s2col[:, jq + q:jq + q + 1])

                # ---- phase 2: Q2C-critical reductions
                pmax = sp.tile([128, CHUNK], f32, tag="pmax")
                psm = sp.tile([128, CHUNK], f32r, tag="psm")
                nc.vector.tensor_tensor(pmax[:], f(e[:, 0, :]), f(e[:, 1, :]), MAX)
                nc.vector.tensor_tensor(psm[:], f(e[:, 0, :]), f(e[:, 1, :]), ADD)
                for jt in range(2, NJT):
                    nc.vector.tensor_tensor(pmax[:], pmax[:], f(e[:, jt, :]), MAX)
                    nc.vector.tensor_tensor(psm[:], f(psm[:]), f(e[:, jt, :]), ADD)

                # s1[t] rows via PE (w1 column stationary), then transpose
                s1ps = rowpsum.tile([1, CHUNK], f32, tag="row",
                                    name=f"s1ps_{c}")
                for kt in range(NKT):
                    nc.tensor.matmul(s1ps[:], wcol[:, kt, 0:1],
                                     ht[:, kt, cs:ce],
                                     start=(kt == 0), stop=(kt == NKT - 1))
                s1row = sp.tile([1, CHUNK], f32, tag="s1row")
                nc.vector.tensor_copy(s1row[:], s1ps[:])

                hnps = rowpsum.tile([1, D2], f32, tag="row", name=f"hnps_{c}")
                for i in range(4):
                    tt = 4 * c + i
                    # emax: transpose pmax 128-block, reduce over partitions
                    tpm = trpsum.tile([128, 128], f32, tag="tr",
                                      name=f"tpm_{c}_{i}")
                    nc.tensor.transpose(tpm[:], pmax[:, i * 128:(i + 1) * 128],
                                        ident[:])
                    nc.vector.tensor_reduce(emax[:, tt:tt + 1], tpm[:], X, MAX)
                    # dcol: ones-matmul, copied to SBUF
                    dps = trpsum.tile([128, 2], f32, tag="dcol",
                                      name=f"dps_{c}_{i}")
                    nc.tensor.matmul(dps[:], psm[:, i * 128:(i + 1) * 128],
                                     onescol[:], start=True, stop=True)
                    nc.vector.tensor_copy(dcol[:, tt:tt + 1], dps[:, 0:1])
                    # s1 column + exp
                    ts1 = trpsum.tile([128, 1], f32, tag="tr",
                                      name=f"ts1_{c}_{i}")
                    nc.tensor.transpose(ts1[:], s1row[0:1, i * 128:(i + 1) * 128],
                                        ident[0:1, 0:1])
                    nc.scalar.activation(es1[:, tt:tt + 1], ts1[:], EXP)
                    # bnum = exp(m[t]) = emax * exp(s1)
                    nc.vector.tensor_tensor(bnum[:, tt:tt + 1],
                                            emax[:, tt:tt + 1],
                                            es1[:, tt:tt + 1], MULT)
                    # Q2C numerator: hnps += bnum_tile.T @ H_tile
                    nc.tensor.matmul(hnps[:], bnum[:, tt:tt + 1],
                                     hn[:, tt, :],
                                     start=(i == 0), stop=(i == 3))
                if c == 0:
                    nc.vector.tensor_copy(hnum_sb[:], hnps[:])
                else:
                    nc.vector.tensor_tensor(hnum_sb[:], hnum_sb[:], hnps[:], ADD)

                # launch the real AllReduce as soon as chunk 3's Q2C
                # reductions are done -- overlaps chunk 3's A phase
                if c == NCHUNK - 1:
                    hts = q2c_global()

                # ---- phase 3: C2Q attend + G blocks 0..2,
                # two interleaved PSUM chains
                for ip in range(0, 4, 2):
                    apss = [apsum.tile([128, D2], f32, tag="aps",
                                       name=f"aps_{c}_{ip}_{q}")
                            for q in range(2)]
                    for jt in range(NJT):
                        for q in range(2):
                            i = ip + q
                            nc.tensor.matmul(
                                apss[q][:],
                                e[:, jt, i * 128:(i + 1) * 128],
                                un[:, jt, :],
                                start=(jt == 0), stop=(jt == NJT - 1))
                    for q in range(2):
                        i = ip + q
                        tt = 4 * c + i
                        dinv = gp.tile([128, 1], f32, tag="dinv")
                        nc.vector.reciprocal(dinv[:], dcol[:, tt:tt + 1])
                        a_sb = gp.tile([128, D2], f32, tag="a_sb")
                        nc.vector.tensor_scalar_mul(a_sb[:], apss[q][:],
                                                    dinv[:])
                        ha_sb = gp.tile([128, D2], f32, tag="ha_sb")
                        nc.vector.tensor_tensor(ha_sb[:], f(hn[:, tt, :]),
                                                a_sb[:], MULT)
                        ts_, te_ = tt * 128, (tt + 1) * 128
                        nc.sync.dma_start(G.ap()[ts_:te_, 0:D2],
                                          f(hn[:, tt, :]))
                        nc.sync.dma_start(G.ap()[ts_:te_, D2:2 * D2], a_sb[:])
                        nc.sync.dma_start(G.ap()[ts_:te_, 2 * D2:3 * D2],
                                          ha_sb[:])

            # ---- G block 3: H * h~ (split DVE / GpSimd; gpsimd is free
            # after the collective completes, which gates block 3 anyway)
            for tt in range(NTT):
                hh_sb = gp.tile([128, D2], f32, tag="hh_sb")
                eng = nc.vector if (tt % 2 == 0) else nc.gpsimd
                eng.tensor_tensor(hh_sb[:], f(hn[:, tt, :]), hts[:], MULT)
                nc.sync.dma_start(G.ap()[tt * 128:(tt + 1) * 128, 3 * D2:4 * D2],
                                  hh_sb[:])

    nc.compile()
    return nc


def kernel(H, U, Ws):
    import concourse.mybir as mybir
    from concourse import bass_utils

    H = np.ascontiguousarray(np.asarray(H, dtype=np.float32))
    U = np.ascontiguousarray(np.asarray(U, dtype=np.float32))
    Ws = np.asarray(Ws, dtype=np.float32)

    if "nc" not in _CACHE:
        _CACHE["nc"] = _build_nc()
    nc = _CACHE["nc"]

    mmnp = (mybir.dt.np(mybir.dt.bfloat16) if MM_BF16 else np.float32)

    w1 = Ws[0:D2, 0]
    w2 = Ws[D2:2 * D2, 0]
    w3 = Ws[2 * D2:3 * D2, 0]
    UW = np.ascontiguousarray(U.T * w3[:, None]).astype(mmnp)
    Unc = U.astype(mmnp)
    Wc = np.ascontiguousarray(np.stack([w1, w2], axis=1))  # [512, 2]
    W2b = np.ascontiguousarray(np.broadcast_to(w2, (128, D2)))
    ident = np.eye(128, dtype=np.float32)

    in_maps = []
    for c in range(NCORES):
        Hc = H[c * TC:(c + 1) * TC]
        in_maps.append({
            "HT": np.ascontiguousarray(Hc.T).astype(mmnp),
            "Hn": Hc,
            "Un": Unc,
            "UW": UW,
            "Wc": Wc,
            "W2b": W2b,
            "Id": ident,
            "On": np.ones((1, 128), dtype=np.float32),
            "Oc": np.ones((128, 2), dtype=np.float32),
        })

    res = bass_utils.run_bass_kernel_spmd(
        nc, in_maps, core_ids=list(range(NCORES)))
    LAST["exec_time_ns"] = res.exec_time_ns
    G_full = np.concatenate([res.results[c]["G"] for c in range(NCORES)],
                            axis=0)
    return G_full.astype(np.float32, copy=False)
